# revision 20
# baseline (speedup 1.0000x reference)
"""Distributed GATv1 (2x GAT + SAGE + MLP head) for Trainium2, 8 NeuronCores.

Strategy (graph/data parallel):
- Nodes sharded contiguously across 8 cores; each core's nodes re-binned into
  128-row tiles balanced by in-degree.
- All heavy tensors are bf16 (gather tables, collectives, matmul inputs):
  4x PE throughput, 2x DVE throughput, half the DMA/collective bytes.
- Layer-1 dense (x @ W1) is computed REPLICATED on every core (cheap: one
  matmul per 128-node tile with x^T supplied pre-transposed), eliminating the
  first AllGather entirely. Each core writes the full g1 table locally.
- Per GAT layer the g table row is [h(192) | a_src(3) | a_dst(3)] (198 cols).
  The edge phase gathers g[src] rows by indirect DMA (the dominant cost is
  ~1.5us of serialized SWDGE descriptor generation per 128-row call, so call
  count is minimized), routes the dst-side attention terms on the Tensor
  engine via transposed one-hots (no per-edge gather), computes
  w = exp(leaky_relu(a_s + a_d)) with two ACT ops, scales the h columns, and
  aggregates per dst tile with one-hot routing matmuls that also accumulate
  the softmax denominators (w written into cols 192:195 of the rhs).
- Self-loops never enter the gather path: each tile's own rows are loaded
  directly and accumulated with one identity matmul (also guarantees a
  nonzero softmax denominator on padding slots), saving one gather call per
  tile per layer and one bucket column of edge work.
- The layer-2 dense is fused into edge-1's tile loop (no f2 DRAM roundtrip)
  and both AllGathers are split into two tile-aligned halves issued mid-loop,
  partially hiding collective time under the remaining edge tiles.
- SAGE + MLP head collapse into two [192,16] matmuls (all-linear tail):
  y = f3 @ (Wl M1 M2) and yr = f3 @ (Wr M1 M2) + c are computed inside the
  edge-2 tile loop; only y ([N,16] bf16, ~1.6MB) is AllGathered. The SAGE
  mean aggregates y[src] with one-hot matmuls; out = sigmoid(agg/deg + yr).
"""

import numpy as np
import ml_dtypes

BF16 = ml_dtypes.bfloat16

# Problem constants (hardcoded; kernel.py must be self-contained).
N = 50000
E = 800000
IN_C = 128
HID = 64
HEADS = 3
OUT_C = 16
C = HEADS * HID          # 192
ROWW = C + 2 * HEADS     # 198 = [h | a_s | a_d]
NCORES = 8
P = 128


def _ceil(a, b):
    return -(-a // b)


def _pack_bins(deg, nbins):
    """Greedy balanced binning: assign n=nbins*128 nodes to bins of 128 slots,
    minimizing the max per-bin edge count. Returns (bin_of, slot_of)."""
    n = len(deg)
    assert n == nbins * P
    order = np.argsort(-deg, kind="stable")
    bin_load = np.zeros(nbins, np.int64)
    bin_fill = np.zeros(nbins, np.int64)
    bin_of = np.zeros(n, np.int32)
    slot_of = np.zeros(n, np.int32)
    big = np.int64(1 << 60)
    for l in order:
        cand = np.where(bin_fill < P, bin_load, big)
        b = int(np.argmin(cand))
        bin_of[l] = b
        slot_of[l] = bin_fill[b]
        bin_fill[b] += 1
        bin_load[b] += deg[l]
    assert (bin_fill == P).all()
    # refinement: swap nodes between bins until every bin load fits the
    # smallest column count (avg rounded up to a multiple of P), trimming
    # one padded gather call per tile when the greedy pass overshoots.
    target = _ceil(_ceil(int(deg.sum()), nbins), P) * P
    loads = np.bincount(bin_of, weights=deg.astype(np.float64),
                        minlength=nbins).astype(np.int64)
    for _ in range(2000):
        hi = int(np.argmax(loads))
        if loads[hi] <= target:
            break
        lo = int(np.argmin(loads))
        ih = np.where(bin_of == hi)[0]
        il = np.where(bin_of == lo)[0]
        delta = deg[ih][:, None] - deg[il][None, :]
        ok = (delta > 0) & (loads[lo] + delta <= target)
        if not ok.any():
            ok = (delta > 0) & (delta < (loads[hi] - loads[lo]))
            if not ok.any():
                break
        want = loads[hi] - target
        score = np.where(ok, -np.abs(delta - want), -(1 << 40))
        u, v = np.unravel_index(np.argmax(score), delta.shape)
        bu, bv = ih[u], il[v]
        bin_of[bu], bin_of[bv] = lo, hi
        d = int(delta[u, v])
        loads[hi] -= d
        loads[lo] += d
    # reassign slots = position within (possibly reshuffled) bins
    fill = np.zeros(nbins, np.int64)
    for l in range(n):
        b = bin_of[l]
        slot_of[l] = fill[b]
        fill[b] += 1
    return bin_of, slot_of


def _bucket_edges(e_dstperm, nbins, cols):
    """Bucket edges by dst bin into [nbins, P, T] arrays (T = max needed).
    cols: list of (array, fill_value, dtype). Returns (T, [out arrays])."""
    ebin = e_dstperm // P
    eslot = (e_dstperm % P).astype(np.float32)
    counts = np.bincount(ebin, minlength=nbins)
    T = max(1, _ceil(int(counts.max()), P))
    order = np.argsort(ebin, kind="stable")
    starts = np.zeros(nbins + 1, np.int64)
    starts[1:] = np.cumsum(counts)
    outs = []
    for arr, fill, dt in cols:
        o = np.full((nbins, P * T), fill, dt)
        for t in range(nbins):
            sel = order[starts[t]:starts[t + 1]]
            o[t, :len(sel)] = arr[sel]
        outs.append(o.reshape(nbins, P, T))
    return T, outs, eslot


def preprocess(x, edge_index, n_nodes, n_cores):
    """Host-side index preprocessing. Returns (cfg dict, per-core data, ggid)."""
    src = np.asarray(edge_index[0], np.int64)
    dst = np.asarray(edge_index[1], np.int64)
    NPC = n_nodes // n_cores
    NPpad = _ceil(NPC, P) * P
    NT = NPpad // P
    NG = n_cores * NPpad

    x = np.asarray(x, np.float32)
    owner = dst // NPC
    deg = np.bincount(dst, minlength=n_nodes).astype(np.int64)

    ggid = np.zeros(n_nodes, np.int64)
    for k in range(n_cores):
        lo, hi = k * NPC, (k + 1) * NPC
        degs = np.concatenate([deg[lo:hi], np.zeros(NPpad - NPC, np.int64)])
        b, s = _pack_bins(degs, NT)
        ggid[lo:hi] = k * NPpad + b[:NPC].astype(np.int64) * P + s[:NPC]

    # x permuted + transposed, replicated to every core
    x_perm = np.zeros((NG, IN_C), np.float32)
    x_perm[ggid] = x
    xT = np.ascontiguousarray(x_perm.T.astype(BF16))

    per_core_raw = []
    T_gat, T_sage = 1, 1
    for k in range(n_cores):
        m = owner == k
        es, ed = src[m], dst[m]
        g_src = ggid[es]
        g_dstg = ggid[ed]
        g_dstl = g_dstg - k * NPpad
        s_src = ggid[es]
        s_dstl = ggid[ed] - k * NPpad
        per_core_raw.append((g_src, g_dstg, g_dstl, s_src, s_dstl))
        T_gat = max(T_gat, _ceil(int(np.bincount(g_dstl // P, minlength=NT).max()), P))

    cores = []
    for k in range(n_cores):
        g_src, g_dstg, g_dstl, s_src, s_dstl = per_core_raw[k]
        # local-first basis for core k: own rank block first, then others in
        # rank order. r_k(g) = blockpos(g//NP)*NP + g%NP.
        bp = np.empty(n_cores, np.int64)
        bp[k] = 0
        others = [r for r in range(n_cores) if r != k]
        for i, r in enumerate(others):
            bp[r] = i + 1
        rk = bp[g_src // NPpad] * NPpad + g_src % NPpad   # src in local-first basis

        # meta2 basis matches the two-half AllGather layout: rank blocks of
        # the first NT_A tiles, then rank blocks of the rest.
        NT_A = (NT + 1) // 2
        HA = NT_A * P
        r_ = g_src // NPpad
        q_ = g_src % NPpad
        g2row = np.where(q_ < HA, r_ * HA + q_,
                         n_cores * HA + r_ * (NPpad - HA) + (q_ - HA))
        Tg, (src1_a, src2_a), gslot = _bucket_edges(
            g_dstl, NT, [(rk, 0, np.int64), (g2row, 0, np.int64)])
        _, (slot_a,), _ = _bucket_edges(g_dstl, NT, [(gslot, -1.0, np.float32)])


        def pad_to(a, T, fill):
            if a.shape[2] < T:
                extra = np.full((NT, P, T - a.shape[2]), fill, a.dtype)
                return np.concatenate([a, extra], 2)
            return a

        src1_a = pad_to(src1_a, T_gat, 0)
        src2_a = pad_to(src2_a, T_gat, 0)
        slot_a = pad_to(slot_a, T_gat, -1.0)

        # slot row layout [NT, 1, Tg*P] for the partition-broadcast matmul
        slot_r = np.ascontiguousarray(
            slot_a.transpose(0, 2, 1).reshape(NT, 1, -1).astype(BF16))

        degs = np.bincount(s_dstl, minlength=NPpad).astype(np.float32)
        deginv = (1.0 / np.maximum(degs, 1.0)).reshape(NT, P, 1)

        # xT in core-k local-first column order
        order = np.concatenate(
            [np.arange(r * NPpad, (r + 1) * NPpad) for r in [k] + others])
        xTk = np.ascontiguousarray(xT[:, order])

        cores.append(dict(
            xT=xTk,
            meta_gat1=np.ascontiguousarray(src1_a.astype(np.int32)),
            meta_gat2=np.ascontiguousarray(src2_a.astype(np.int32)),
            slot_gat=np.ascontiguousarray(slot_a.astype(BF16)),
            slot_gat_r=slot_r,
            sdeginv=np.ascontiguousarray(deginv.astype(np.float32)),
        ))

    cfg = dict(n_cores=n_cores, NPC=NPC, NP=NPpad, NT=NT,
               T_gat=T_gat, T_sage=T_gat, Fin=x.shape[1])
    return cfg, cores, ggid


def fold_weights(W1, a1s, a1d, b1, W2, a2s, a2d, b2, Wl, bl, Wr, M1, mb1, M2, mb2):
    """Host-side weight folding -> replicated device weight arrays (bf16)."""
    f = lambda a: np.asarray(a, np.float32)
    W1, a1s, a1d, b1 = f(W1), f(a1s), f(a1d), f(b1)
    W2, a2s, a2d, b2 = f(W2), f(a2s), f(a2d), f(b2)
    Wl, bl, Wr, M1, mb1, M2, mb2 = f(Wl), f(bl), f(Wr), f(M1), f(mb1), f(M2), f(mb2)

    def bd(a):  # [HEADS, HID] -> block diag [C, HEADS]
        out = np.zeros((C, HEADS), np.float32)
        for h in range(HEADS):
            out[h * HID:(h + 1) * HID, h] = a[h]
        return out

    w1cat = np.concatenate([W1, W1 @ bd(a1s), W1 @ bd(a1d)], 1)  # [Fin,198]
    w2cat = np.concatenate([W2, W2 @ bd(a2s), W2 @ bd(a2d)], 1)  # [C,198]
    wlmm = Wl @ M1 @ M2                                          # [C,16]
    wrmm = Wr @ M1 @ M2                                          # [C,16]
    cvec = bl @ M1 @ M2 + mb1 @ M2 + mb2                         # [16]
    bfc = lambda a: np.ascontiguousarray(a.astype(BF16))
    return dict(
        w1cat=bfc(w1cat),
        w2a=bfc(w2cat[0:P]),
        w2b=bfc(w2cat[P:C]),
        wla=bfc(wlmm[0:P]),
        wlb=bfc(wlmm[P:C]),
        wra=bfc(wrmm[0:P]),
        wrb=bfc(wrmm[P:C]),
        brep1=bfc(np.tile(b1[None, :], (P, 1))),
        brep2=bfc(np.tile(b2[None, :], (P, 1))),
        crep=np.ascontiguousarray(np.tile(cvec[None, :], (P, 1)).astype(np.float32)),
    )


def build_program(cfg, reps=1):
    """Build the Bass/Tile program (SPMD, identical across cores).

    reps>1 repeats the whole computation (for floor-free benchmarking)."""
    import concourse.bass as bass
    import concourse.bacc as bacc
    import concourse.mybir as mybir
    import concourse.tile as tile
    from concourse.masks import make_identity

    n_cores = cfg["n_cores"]
    NP_, NT_, Tg, Ts, Fin = cfg["NP"], cfg["NT"], cfg["T_gat"], cfg["T_sage"], cfg["Fin"]
    NG = n_cores * NP_
    f32 = mybir.dt.float32
    i32 = mybir.dt.int32
    bf = mybir.dt.bfloat16
    A = mybir.AluOpType
    ACT = mybir.ActivationFunctionType

    _ceil_i = lambda a, b: -(-a // b)

    nc = bacc.Bacc("TRN2", target_bir_lowering=False, num_devices=n_cores)

    # I/O
    xT = nc.dram_tensor("xT", [P, NG], bf, kind="ExternalInput")
    w1cat = nc.dram_tensor("w1cat", [Fin, ROWW], bf, kind="ExternalInput")
    w2a = nc.dram_tensor("w2a", [P, ROWW], bf, kind="ExternalInput")
    w2b = nc.dram_tensor("w2b", [C - P, ROWW], bf, kind="ExternalInput")
    wla = nc.dram_tensor("wla", [P, OUT_C], bf, kind="ExternalInput")
    wlb = nc.dram_tensor("wlb", [C - P, OUT_C], bf, kind="ExternalInput")
    wra = nc.dram_tensor("wra", [P, OUT_C], bf, kind="ExternalInput")
    wrb = nc.dram_tensor("wrb", [C - P, OUT_C], bf, kind="ExternalInput")
    brep1 = nc.dram_tensor("brep1", [P, C], bf, kind="ExternalInput")
    brep2 = nc.dram_tensor("brep2", [P, C], bf, kind="ExternalInput")
    crep = nc.dram_tensor("crep", [P, OUT_C], f32, kind="ExternalInput")
    meta_gat1 = nc.dram_tensor("meta_gat1", [NT_, P, Tg], i32, kind="ExternalInput")
    meta_gat2 = nc.dram_tensor("meta_gat2", [NT_, P, Tg], i32, kind="ExternalInput")
    slot_gat = nc.dram_tensor("slot_gat", [NT_, P, Tg], bf, kind="ExternalInput")
    slot_gat_r = nc.dram_tensor("slot_gat_r", [NT_, 1, Tg * P], bf,
                                kind="ExternalInput")
    sdeginv = nc.dram_tensor("sdeginv", [NT_, P, 1], f32, kind="ExternalInput")
    out_sh = nc.dram_tensor("out_sh", [NP_, OUT_C], f32, kind="ExternalOutput")

    g1_full = nc.dram_tensor("g1_full", [NG, ROWW], bf, kind="Internal")
    ald1_loc = nc.dram_tensor("ald1_loc", [NP_, 4], bf, kind="Internal")
    g2_loc = nc.dram_tensor("g2_loc", [NP_, ROWW], bf, kind="Internal")
    ald2_loc = nc.dram_tensor("ald2_loc", [NP_, 4], bf, kind="Internal")
    y_loc = nc.dram_tensor("y_loc", [NP_, OUT_C], bf, kind="Internal")
    yr_loc = nc.dram_tensor("yr_loc", [NP_, OUT_C], f32, kind="Internal")
    if n_cores > 1:
        g2_full = nc.dram_tensor("g2_full", [NG, ROWW], bf, kind="Internal",
                                 addr_space="Shared")
        y_full = nc.dram_tensor("y_full", [NG, OUT_C], bf, kind="Internal",
                                addr_space="Shared")
    else:
        g2_full, y_full = g2_loc, y_loc

    with tile.TileContext(nc) as tc:
        import contextlib
        ctx = contextlib.ExitStack()
        with ctx:
            cpool = ctx.enter_context(tc.tile_pool(name="const", bufs=1))
            dpool = ctx.enter_context(tc.tile_pool(name="dense", bufs=4))
            epool = ctx.enter_context(tc.tile_pool(name="edge", bufs=4))
            spool = ctx.enter_context(tc.tile_pool(name="spool", bufs=3))
            accps = ctx.enter_context(tc.tile_pool(name="accps", bufs=2, space="PSUM"))
            trps = ctx.enter_context(tc.tile_pool(name="trps", bufs=1, space="PSUM"))
            brps_p = ctx.enter_context(tc.tile_pool(name="brps", bufs=2, space="PSUM"))
            aldps_p = ctx.enter_context(tc.tile_pool(name="aldps", bufs=1, space="PSUM"))
            ops_ps = ctx.enter_context(tc.tile_pool(name="opsps", bufs=1, space="PSUM"))

            # constants
            iota_i = cpool.tile([P, P], i32)
            iota_b = cpool.tile([P, P], bf)
            nc.gpsimd.iota(iota_i[:], pattern=[[1, P]], base=0, channel_multiplier=0)
            nc.vector.tensor_copy(iota_b[:], iota_i[:])
            ident = cpool.tile([P, P], bf)
            make_identity(nc, ident[:])
            ones_sb = cpool.tile([1, P], bf)
            nc.vector.memset(ones_sb[:], 1.0)
            ipt_i = cpool.tile([P, 1], i32)
            ipt_b = cpool.tile([P, 1], bf)
            nc.gpsimd.iota(ipt_i[:], pattern=[[0, 1]], base=0, channel_multiplier=1)
            nc.vector.tensor_copy(ipt_b[:], ipt_i[:])

            w1sb = cpool.tile([Fin, ROWW], bf)
            nc.sync.dma_start(w1sb[:], w1cat[:, :])
            w2a_sb = cpool.tile([P, ROWW], bf)
            w2b_sb = cpool.tile([C - P, ROWW], bf)
            nc.sync.dma_start(w2a_sb[:], w2a[:, :])
            nc.sync.dma_start(w2b_sb[:], w2b[:, :])
            wla_sb = cpool.tile([P, OUT_C], bf)
            wlb_sb = cpool.tile([C - P, OUT_C], bf)
            wra_sb = cpool.tile([P, OUT_C], bf)
            wrb_sb = cpool.tile([C - P, OUT_C], bf)
            nc.sync.dma_start(wla_sb[:], wla[:, :])
            nc.sync.dma_start(wlb_sb[:], wlb[:, :])
            nc.sync.dma_start(wra_sb[:], wra[:, :])
            nc.sync.dma_start(wrb_sb[:], wrb[:, :])
            b1sb = cpool.tile([P, C], bf)
            b2sb = cpool.tile([P, C], bf)
            csb = cpool.tile([P, OUT_C], f32)
            nc.sync.dma_start(b1sb[:], brep1[:, :])
            nc.sync.dma_start(b2sb[:], brep2[:, :])
            nc.sync.dma_start(csb[:], crep[:, :])

            def dense1(scope):
                # replicated: g1 rows for ALL cores' nodes, written locally.
                # GS tiles per iteration to amortize DMA fixed costs.
                GS = 8
                with nc.named_scope(scope):
                    for c0 in range(0, NG // P, GS):
                        gs = min(GS, NG // P - c0)
                        xt = dpool.tile([P, GS * P], bf, tag="xt")
                        nc.sync.dma_start(xt[:, 0:gs * P],
                                          xT[:, c0 * P:(c0 + gs) * P])
                        gsb = dpool.tile([P, GS, ROWW], bf, tag="gsb")
                        n_ald = max(0, min(gs, NT_ - c0))
                        asb = None
                        if n_ald:
                            asb = dpool.tile([P, GS, 4], bf, tag="asb")
                        for i in range(gs):
                            ps = accps.tile([P, ROWW], f32, tag="acc")
                            nc.tensor.matmul(out=ps[:],
                                             lhsT=xt[:, i * P:(i + 1) * P],
                                             rhs=w1sb[:], start=True, stop=True)
                            nc.scalar.activation(gsb[:, i, :], ps[:], ACT.Copy)
                            if i < n_ald:
                                nc.scalar.activation(asb[:, i, :],
                                                     ps[:, C + 2:ROWW], ACT.Copy)
                        nc.sync.dma_start(
                            g1_full[c0 * P:(c0 + gs) * P, :].rearrange(
                                "(i p) w -> p i w", p=P),
                            gsb[:, 0:gs, :])
                        if n_ald:
                            nc.sync.dma_start(
                                ald1_loc[c0 * P:(c0 + n_ald) * P, :].rearrange(
                                    "(i p) w -> p i w", p=P),
                                asb[:, 0:n_ald, :])

            def edge_phase(meta_dram, g_dram, ald_dram, self_dram, b_sb, fused,
                           scope, mid_hook=None):
                TgP = Tg * P
                NB = _ceil_i(TgP, 512)
                with nc.named_scope(scope):
                    for t in range(NT_):
                        mi = epool.tile([P, Tg], i32, tag="mi")
                        nc.sync.dma_start(mi[:], meta_dram[t, :, :])
                        slt = epool.tile([P, Tg], bf, tag="slt")
                        nc.sync.dma_start(slt[:], slot_gat[t, :, :])
                        slr = epool.tile([1, TgP], bf, tag="slr")
                        nc.sync.dma_start(slr[:], slot_gat_r[t, :, :])
                        aldt = epool.tile([P, 4], bf, tag="aldt")
                        nc.sync.dma_start(aldt[:], ald_dram[t * P:(t + 1) * P, :])
                        G = epool.tile([P, Tg, ROWW], bf, tag="G")
                        for j in range(Tg):
                            nc.gpsimd.indirect_dma_start(
                                out=G[:, j, :], out_offset=None, in_=g_dram[:, :],
                                in_offset=bass.IndirectOffsetOnAxis(
                                    ap=mi[:, j:j + 1], axis=0))
                        S_all = spool.tile([P, Tg, P], bf, tag="S")
                        nc.vector.tensor_tensor(
                            out=S_all[:],
                            in0=iota_b[:].unsqueeze(1).broadcast_to([P, Tg, P]),
                            in1=slt[:].unsqueeze(2).broadcast_to([P, Tg, P]),
                            op=A.is_equal)
                        # dst-side attention terms routed via transposed one-hots:
                        # st_all[d, j*P+e] = (slot_e == d); alde = st^T @ aldt
                        st_all = spool.tile([P, TgP], bf, tag="st")
                        for g in range(NB):
                            lo = g * 512
                            hi = min(lo + 512, TgP)
                            brt = brps_p.tile([P, 512], f32, tag="br")
                            nc.tensor.matmul(out=brt[:, 0:hi - lo], lhsT=ones_sb[:],
                                             rhs=slr[:, lo:hi], start=True, stop=True)
                            brb = epool.tile([P, 512], bf, tag="brb")
                            nc.scalar.activation(brb[:, 0:hi - lo], brt[:, 0:hi - lo],
                                                 ACT.Copy)
                            nc.vector.tensor_tensor(
                                out=st_all[:, lo:hi],
                                in0=ipt_b[:].broadcast_to([P, hi - lo]),
                                in1=brb[:, 0:hi - lo], op=A.is_equal)
                        aldeps = aldps_p.tile([P, Tg * HEADS], f32, tag="alde")
                        for j in range(Tg):
                            nc.tensor.matmul(
                                out=aldeps[:, j * HEADS:(j + 1) * HEADS],
                                lhsT=st_all[:, j * P:(j + 1) * P],
                                rhs=aldt[:, 1:4], start=True, stop=True)
                        ab = epool.tile([P, Tg, HEADS], bf, tag="ab")
                        nc.scalar.activation(
                            ab[:].rearrange("p j h -> p (j h)"), aldeps[:], ACT.Copy)
                        att = epool.tile([P, Tg, HEADS], bf, tag="att")
                        nc.vector.tensor_tensor(out=att[:], in0=G[:, :, C:C + HEADS],
                                                in1=ab[:], op=A.add)
                        nc.scalar.activation(att[:], att[:], ACT.Lrelu, alpha=0.2)
                        nc.scalar.activation(att[:], att[:], ACT.Exp)
                        nc.vector.tensor_copy(G[:, :, C:C + HEADS], att[:])
                        gh = G[:, :, 0:C].rearrange("p j (h c) -> p j h c", h=HEADS)
                        nc.vector.tensor_tensor(
                            out=gh, in0=gh,
                            in1=att[:].to_broadcast([P, Tg, HEADS, HID]),
                            op=A.mult)
                        # self-loop contribution: rhs built from the tile's own
                        # rows (direct load), accumulated via an identity matmul
                        gt_self = epool.tile([P, ROWW], bf, tag="gself")
                        nc.sync.dma_start(gt_self[:],
                                          self_dram[t * P:(t + 1) * P, :])
                        att_s = epool.tile([P, HEADS], bf, tag="atts")
                        nc.vector.tensor_tensor(out=att_s[:],
                                                in0=gt_self[:, C:C + HEADS],
                                                in1=aldt[:, 1:4], op=A.add)
                        nc.scalar.activation(att_s[:], att_s[:], ACT.Lrelu, alpha=0.2)
                        nc.scalar.activation(att_s[:], att_s[:], ACT.Exp)
                        selfrhs = epool.tile([P, ROWW], bf, tag="selfrhs")
                        nc.vector.tensor_tensor(
                            out=selfrhs[:, 0:C].rearrange("p (h c) -> p h c", h=HEADS),
                            in0=gt_self[:, 0:C].rearrange("p (h c) -> p h c", h=HEADS),
                            in1=att_s[:].to_broadcast([P, HEADS, HID]), op=A.mult)
                        nc.vector.tensor_copy(selfrhs[:, C:C + HEADS], att_s[:])
                        nc.vector.tensor_copy(selfrhs[:, C + HEADS:ROWW], att_s[:])
                        ps = accps.tile([P, ROWW], f32, tag="acc")
                        for j in range(Tg):
                            nc.tensor.matmul(out=ps[:], lhsT=S_all[:, j, :],
                                             rhs=G[:, j, :],
                                             start=(j == 0), stop=False)
                        nc.tensor.matmul(out=ps[:], lhsT=ident[:], rhs=selfrhs[:],
                                         start=False, stop=True)
                        zinv = epool.tile([P, HEADS], f32, tag="zinv")
                        nc.vector.reciprocal(zinv[:], ps[:, C:C + HEADS])
                        osb = epool.tile([P, C], bf, tag="osb")
                        nc.vector.tensor_tensor(
                            out=osb[:].rearrange("p (h c) -> p h c", h=HEADS),
                            in0=ps[:, 0:C].rearrange("p (h c) -> p h c", h=HEADS),
                            in1=zinv[:].to_broadcast([P, HEADS, HID]),
                            op=A.mult)
                        nc.vector.tensor_tensor(out=osb[:], in0=osb[:], in1=b_sb[:],
                                                op=A.add)
                        if not fused:
                            f2sb = epool.tile([P, C], bf, tag="fout")
                            nc.scalar.activation(f2sb[:], osb[:], ACT.Relu)
                            tp1 = trps.tile([P, P], bf, tag="tp")
                            nc.tensor.transpose(out=tp1[:], in_=f2sb[:, 0:P],
                                                identity=ident[:])
                            ft1 = epool.tile([P, P], bf, tag="ft1")
                            nc.scalar.activation(ft1[:], tp1[:], ACT.Copy)
                            tp2 = trps.tile([P, P], bf, tag="tp")
                            nc.tensor.transpose(out=tp2[:C - P, :], in_=f2sb[:, P:C],
                                                identity=ident[:])
                            ft2 = epool.tile([C - P, P], bf, tag="ft2")
                            nc.scalar.activation(ft2[:], tp2[:C - P, :], ACT.Copy)
                            ps2 = accps.tile([P, ROWW], f32, tag="acc")
                            nc.tensor.matmul(out=ps2[:], lhsT=ft1[:], rhs=w2a_sb[:],
                                             start=True, stop=False)
                            nc.tensor.matmul(out=ps2[:], lhsT=ft2[:], rhs=w2b_sb[:],
                                             start=False, stop=True)
                            gsb = epool.tile([P, ROWW], bf, tag="g2sb")
                            nc.scalar.activation(gsb[:], ps2[:], ACT.Copy)
                            nc.sync.dma_start(g2_loc[t * P:(t + 1) * P, :], gsb[:])
                            asb = epool.tile([P, 4], bf, tag="a2sb")
                            nc.scalar.activation(asb[:], ps2[:, C + 2:ROWW], ACT.Copy)
                            nc.sync.dma_start(ald2_loc[t * P:(t + 1) * P, :], asb[:])
                        else:
                            f3t = epool.tile([P, C], bf, tag="fout")
                            nc.scalar.activation(f3t[:], osb[:], ACT.Relu)
                            tp1 = trps.tile([P, P], bf, tag="tp")
                            nc.tensor.transpose(out=tp1[:], in_=f3t[:, 0:P],
                                                identity=ident[:])
                            ft1 = epool.tile([P, P], bf, tag="ft1")
                            nc.scalar.activation(ft1[:], tp1[:], ACT.Copy)
                            tp2 = trps.tile([P, P], bf, tag="tp")
                            nc.tensor.transpose(out=tp2[:C - P, :], in_=f3t[:, P:C],
                                                identity=ident[:])
                            ft2 = epool.tile([C - P, P], bf, tag="ft2")
                            nc.scalar.activation(ft2[:], tp2[:C - P, :], ACT.Copy)
                            yps = ops_ps.tile([P, OUT_C], f32, tag="yps")
                            nc.tensor.matmul(out=yps[:], lhsT=ft1[:], rhs=wla_sb[:],
                                             start=True, stop=False)
                            nc.tensor.matmul(out=yps[:], lhsT=ft2[:], rhs=wlb_sb[:],
                                             start=False, stop=True)
                            yrps = ops_ps.tile([P, OUT_C], f32, tag="yrps")
                            nc.tensor.matmul(out=yrps[:], lhsT=ft1[:], rhs=wra_sb[:],
                                             start=True, stop=False)
                            nc.tensor.matmul(out=yrps[:], lhsT=ft2[:], rhs=wrb_sb[:],
                                             start=False, stop=True)
                            ysb = epool.tile([P, OUT_C], bf, tag="ysb")
                            nc.scalar.activation(ysb[:], yps[:], ACT.Copy)
                            nc.sync.dma_start(y_loc[t * P:(t + 1) * P, :], ysb[:])
                            yrsb = epool.tile([P, OUT_C], f32, tag="yrsb")
                            nc.vector.tensor_tensor(out=yrsb[:], in0=yrps[:],
                                                    in1=csb[:], op=A.add)
                            nc.sync.dma_start(yr_loc[t * P:(t + 1) * P, :], yrsb[:])
                        if mid_hook is not None:
                            mid_hook(t)

            def allgather(loc_ap, full_ap, scope):
                with nc.named_scope(scope):
                    nc.gpsimd.collective_compute(
                        "AllGather", A.bypass,
                        replica_groups=[list(range(n_cores))],
                        ins=[loc_ap],
                        outs=[full_ap],
                    )

            def sage(scope):
                with nc.named_scope(scope):
                    for t in range(NT_):
                        mi = epool.tile([P, Ts], i32, tag="smi")
                        nc.sync.dma_start(mi[:], meta_gat2[t, :, :])
                        slt = epool.tile([P, Ts], bf, tag="sslt")
                        nc.sync.dma_start(slt[:], slot_gat[t, :, :])
                        dg = epool.tile([P, 1], f32, tag="dg")
                        nc.sync.dma_start(dg[:], sdeginv[t, :, :])
                        Y = epool.tile([P, Ts, OUT_C], bf, tag="Y")
                        for j in range(Ts):
                            nc.gpsimd.indirect_dma_start(
                                out=Y[:, j, :], out_offset=None, in_=y_full[:, :],
                                in_offset=bass.IndirectOffsetOnAxis(
                                    ap=mi[:, j:j + 1], axis=0))
                        S_s = spool.tile([P, Ts, P], bf, tag="Ss")
                        nc.vector.tensor_tensor(
                            out=S_s[:],
                            in0=iota_b[:].unsqueeze(1).broadcast_to([P, Ts, P]),
                            in1=slt[:].unsqueeze(2).broadcast_to([P, Ts, P]),
                            op=A.is_equal)
                        ps = ops_ps.tile([P, OUT_C], f32, tag="yps")
                        for j in range(Ts):
                            nc.tensor.matmul(out=ps[:], lhsT=S_s[:, j, :],
                                             rhs=Y[:, j, :],
                                             start=(j == 0), stop=(j == Ts - 1))
                        agg = epool.tile([P, OUT_C], f32, tag="agg")
                        nc.vector.tensor_scalar(out=agg[:], in0=ps[:],
                                                scalar1=dg[:], scalar2=None,
                                                op0=A.mult)
                        yrt = epool.tile([P, OUT_C], f32, tag="yrt")
                        nc.sync.dma_start(yrt[:], yr_loc[t * P:(t + 1) * P, :])
                        nc.vector.tensor_tensor(out=agg[:], in0=agg[:], in1=yrt[:],
                                                op=A.add)
                        fin = epool.tile([P, OUT_C], f32, tag="fin")
                        nc.scalar.activation(fin[:], agg[:], ACT.Sigmoid)
                        nc.sync.dma_start(out_sh[t * P:(t + 1) * P, :], fin[:])

            # ---- program ----
            NT_A = (NT_ + 1) // 2
            HA = NT_A * P
            NG_A = n_cores * HA

            for rep in range(reps):
                r = f"_r{rep}" if reps > 1 else ""

                def ag2_hook(t, _r=r):
                    if n_cores <= 1:
                        return
                    if t == NT_A - 1:
                        allgather(g2_loc[0:HA, :], g2_full[0:NG_A, :], f"ag2a{_r}")
                    elif t == NT_ - 1:
                        allgather(g2_loc[HA:NP_, :], g2_full[NG_A:NG, :], f"ag2b{_r}")

                def agy_hook(t, _r=r):
                    if n_cores <= 1:
                        return
                    if t == NT_A - 1:
                        allgather(y_loc[0:HA, :], y_full[0:NG_A, :], f"agya{_r}")
                    elif t == NT_ - 1:
                        allgather(y_loc[HA:NP_, :], y_full[NG_A:NG, :], f"agyb{_r}")

                dense1(f"dense1{r}")
                edge_phase(meta_gat1, g1_full, ald1_loc, g1_full, b1sb, False,
                           f"edge1{r}", mid_hook=ag2_hook)
                edge_phase(meta_gat2, g2_full, ald2_loc, g2_loc, b2sb, True,
                           f"edge2{r}", mid_hook=agy_hook)
                sage(f"sage{r}")

    nc.compile()
    return nc


LAST_RESULTS = None


def kernel(**inputs):
    global LAST_RESULTS
    import os
    x = np.asarray(inputs["x"], np.float32)
    edge_index = np.asarray(inputs["edge_index"])
    cfg, cores, ggid = preprocess(x, edge_index, N, NCORES)
    wts = fold_weights(
        inputs["W1"], inputs["a1s"], inputs["a1d"], inputs["b1"],
        inputs["W2"], inputs["a2s"], inputs["a2d"], inputs["b2"],
        inputs["Wl"], inputs["bl"], inputs["Wr"],
        inputs["M1"], inputs["mb1"], inputs["M2"], inputs["mb2"])
    nc = build_program(cfg)
    in_maps = [dict(core, **wts) for core in cores]

    from concourse import bass_utils
    res = bass_utils.run_bass_kernel_spmd(
        nc, in_maps, core_ids=list(range(NCORES)),
        trace=bool(int(os.environ.get("GAT_TRACE", "0"))))
    LAST_RESULTS = res
    NPp = cfg["NP"]
    out = np.zeros((N, OUT_C), np.float32)
    for k in range(NCORES):
        o = res.results[k]["out_sh"]
        lo, hi = k * cfg["NPC"], (k + 1) * cfg["NPC"]
        out[lo:hi] = o[ggid[lo:hi] - k * NPp]
    return out


# revision 22
# speedup vs baseline: 1.4705x; 1.4705x over previous
"""Distributed GATv1 (2x GAT + SAGE + MLP head) for Trainium2, 8 NeuronCores.

Strategy (graph/data parallel):
- Nodes sharded contiguously across 8 cores; each core's nodes re-binned into
  128-row tiles balanced by in-degree.
- All heavy tensors are bf16 (gather tables, collectives, matmul inputs):
  4x PE throughput, 2x DVE throughput, half the DMA/collective bytes.
- Layer-1 dense (x @ W1) is computed REPLICATED on every core (cheap: one
  matmul per 128-node tile with x^T supplied pre-transposed), eliminating the
  first AllGather entirely. Each core writes the full g1 table locally.
- Per GAT layer the g table row is [h(192) | a_src(3) | a_dst(3)] (198 cols).
  The edge phase gathers g[src] rows by indirect DMA (the dominant cost is
  ~1.5us of serialized SWDGE descriptor generation per 128-row call, so call
  count is minimized), routes the dst-side attention terms on the Tensor
  engine via transposed one-hots (no per-edge gather), computes
  w = exp(leaky_relu(a_s + a_d)) with two ACT ops, scales the h columns, and
  aggregates per dst tile with one-hot routing matmuls that also accumulate
  the softmax denominators (w written into cols 192:195 of the rhs).
- Self-loops never enter the gather path: each tile's own rows are loaded
  directly and accumulated with one identity matmul (also guarantees a
  nonzero softmax denominator on padding slots), saving one gather call per
  tile per layer and one bucket column of edge work.
- The layer-2 dense is fused into edge-1's tile loop (no f2 DRAM roundtrip)
  and both AllGathers are split into two tile-aligned halves issued mid-loop,
  partially hiding collective time under the remaining edge tiles.
- SAGE + MLP head collapse into two [192,16] matmuls (all-linear tail):
  y = f3 @ (Wl M1 M2) and yr = f3 @ (Wr M1 M2) + c are computed inside the
  edge-2 tile loop; only y ([N,16] bf16, ~1.6MB) is AllGathered. The SAGE
  mean aggregates y[src] with one-hot matmuls; out = sigmoid(agg/deg + yr).
"""

import numpy as np
import ml_dtypes

BF16 = ml_dtypes.bfloat16

# Problem constants (hardcoded; kernel.py must be self-contained).
N = 50000
E = 800000
IN_C = 128
HID = 64
HEADS = 3
OUT_C = 16
C = HEADS * HID          # 192
ROWW = C + 2 * HEADS     # 198 = [h | a_s | a_d]
NCORES = 8
P = 128


def _ceil(a, b):
    return -(-a // b)


def _pack_bins(deg, nbins):
    """Greedy balanced binning: assign n=nbins*128 nodes to bins of 128 slots,
    minimizing the max per-bin edge count. Returns (bin_of, slot_of)."""
    n = len(deg)
    assert n == nbins * P
    order = np.argsort(-deg, kind="stable")
    bin_load = np.zeros(nbins, np.int64)
    bin_fill = np.zeros(nbins, np.int64)
    bin_of = np.zeros(n, np.int32)
    slot_of = np.zeros(n, np.int32)
    big = np.int64(1 << 60)
    for l in order:
        cand = np.where(bin_fill < P, bin_load, big)
        b = int(np.argmin(cand))
        bin_of[l] = b
        slot_of[l] = bin_fill[b]
        bin_fill[b] += 1
        bin_load[b] += deg[l]
    assert (bin_fill == P).all()
    # refinement: swap nodes between bins until every bin load fits the
    # smallest column count (avg rounded up to a multiple of P), trimming
    # one padded gather call per tile when the greedy pass overshoots.
    target = _ceil(_ceil(int(deg.sum()), nbins), P) * P
    loads = np.bincount(bin_of, weights=deg.astype(np.float64),
                        minlength=nbins).astype(np.int64)
    for _ in range(2000):
        hi = int(np.argmax(loads))
        if loads[hi] <= target:
            break
        lo = int(np.argmin(loads))
        ih = np.where(bin_of == hi)[0]
        il = np.where(bin_of == lo)[0]
        delta = deg[ih][:, None] - deg[il][None, :]
        ok = (delta > 0) & (loads[lo] + delta <= target)
        if not ok.any():
            ok = (delta > 0) & (delta < (loads[hi] - loads[lo]))
            if not ok.any():
                break
        want = loads[hi] - target
        score = np.where(ok, -np.abs(delta - want), -(1 << 40))
        u, v = np.unravel_index(np.argmax(score), delta.shape)
        bu, bv = ih[u], il[v]
        bin_of[bu], bin_of[bv] = lo, hi
        d = int(delta[u, v])
        loads[hi] -= d
        loads[lo] += d
    # reassign slots = position within (possibly reshuffled) bins
    fill = np.zeros(nbins, np.int64)
    for l in range(n):
        b = bin_of[l]
        slot_of[l] = fill[b]
        fill[b] += 1
    return bin_of, slot_of


def _bucket_edges(e_dstperm, nbins, cols):
    """Bucket edges by dst bin into [nbins, P, T] arrays (T = max needed).
    cols: list of (array, fill_value, dtype). Returns (T, [out arrays])."""
    ebin = e_dstperm // P
    eslot = (e_dstperm % P).astype(np.float32)
    counts = np.bincount(ebin, minlength=nbins)
    T = max(1, _ceil(int(counts.max()), P))
    order = np.argsort(ebin, kind="stable")
    starts = np.zeros(nbins + 1, np.int64)
    starts[1:] = np.cumsum(counts)
    outs = []
    for arr, fill, dt in cols:
        o = np.full((nbins, P * T), fill, dt)
        for t in range(nbins):
            sel = order[starts[t]:starts[t + 1]]
            o[t, :len(sel)] = arr[sel]
        outs.append(o.reshape(nbins, P, T))
    return T, outs, eslot


def preprocess(x, edge_index, n_nodes, n_cores):
    """Host-side index preprocessing. Returns (cfg dict, per-core data, ggid)."""
    src = np.asarray(edge_index[0], np.int64)
    dst = np.asarray(edge_index[1], np.int64)
    NPC = n_nodes // n_cores
    NPpad = _ceil(NPC, P) * P
    NT = NPpad // P
    NG = n_cores * NPpad

    x = np.asarray(x, np.float32)
    owner = dst // NPC
    deg = np.bincount(dst, minlength=n_nodes).astype(np.int64)

    ggid = np.zeros(n_nodes, np.int64)
    for k in range(n_cores):
        lo, hi = k * NPC, (k + 1) * NPC
        degs = np.concatenate([deg[lo:hi], np.zeros(NPpad - NPC, np.int64)])
        b, s = _pack_bins(degs, NT)
        ggid[lo:hi] = k * NPpad + b[:NPC].astype(np.int64) * P + s[:NPC]

    # x permuted + transposed, replicated to every core
    x_perm = np.zeros((NG, IN_C), np.float32)
    x_perm[ggid] = x
    xT = np.ascontiguousarray(x_perm.T.astype(BF16))

    per_core_raw = []
    T_gat, T_sage = 1, 1
    for k in range(n_cores):
        m = owner == k
        es, ed = src[m], dst[m]
        g_src = ggid[es]
        g_dstg = ggid[ed]
        g_dstl = g_dstg - k * NPpad
        s_src = ggid[es]
        s_dstl = ggid[ed] - k * NPpad
        per_core_raw.append((g_src, g_dstg, g_dstl, s_src, s_dstl))
        T_gat = max(T_gat, _ceil(int(np.bincount(g_dstl // P, minlength=NT).max()), P))

    cores = []
    for k in range(n_cores):
        g_src, g_dstg, g_dstl, s_src, s_dstl = per_core_raw[k]
        # local-first basis for core k: own rank block first, then others in
        # rank order. r_k(g) = blockpos(g//NP)*NP + g%NP.
        bp = np.empty(n_cores, np.int64)
        bp[k] = 0
        others = [r for r in range(n_cores) if r != k]
        for i, r in enumerate(others):
            bp[r] = i + 1
        rk = bp[g_src // NPpad] * NPpad + g_src % NPpad   # src in local-first basis

        # meta2 basis matches the two-half AllGather layout: rank blocks of
        # the first NT_A tiles, then rank blocks of the rest.
        NT_A = (NT + 1) // 2
        HA = NT_A * P
        r_ = g_src // NPpad
        q_ = g_src % NPpad
        g2row = np.where(q_ < HA, r_ * HA + q_,
                         n_cores * HA + r_ * (NPpad - HA) + (q_ - HA))
        Tg, (src1_a, src2_a), gslot = _bucket_edges(
            g_dstl, NT, [(rk, 0, np.int64), (g2row, 0, np.int64)])
        _, (slot_a,), _ = _bucket_edges(g_dstl, NT, [(gslot, -1.0, np.float32)])


        def pad_to(a, T, fill):
            if a.shape[2] < T:
                extra = np.full((NT, P, T - a.shape[2]), fill, a.dtype)
                return np.concatenate([a, extra], 2)
            return a

        src1_a = pad_to(src1_a, T_gat, 0)
        src2_a = pad_to(src2_a, T_gat, 0)
        slot_a = pad_to(slot_a, T_gat, -1.0)

        # slot row layout [NT, 1, Tg*P] for the partition-broadcast matmul
        slot_r = np.ascontiguousarray(
            slot_a.transpose(0, 2, 1).reshape(NT, 1, -1).astype(BF16))

        degs = np.bincount(s_dstl, minlength=NPpad).astype(np.float32)
        deginv = (1.0 / np.maximum(degs, 1.0)).reshape(NT, P, 1)

        # xT in core-k local-first column order
        order = np.concatenate(
            [np.arange(r * NPpad, (r + 1) * NPpad) for r in [k] + others])
        xTk = np.ascontiguousarray(xT[:, order])

        cores.append(dict(
            xT=xTk,
            meta_gat1=np.ascontiguousarray(src1_a.astype(np.int32)),
            meta_gat2=np.ascontiguousarray(src2_a.astype(np.int32)),
            slot_gat=np.ascontiguousarray(slot_a.astype(BF16)),
            slot_gat_r=slot_r,
            sdeginv=np.ascontiguousarray(deginv.astype(np.float32)),
        ))

    cfg = dict(n_cores=n_cores, NPC=NPC, NP=NPpad, NT=NT,
               T_gat=T_gat, T_sage=T_gat, Fin=x.shape[1])
    return cfg, cores, ggid


def fold_weights(W1, a1s, a1d, b1, W2, a2s, a2d, b2, Wl, bl, Wr, M1, mb1, M2, mb2):
    """Host-side weight folding -> replicated device weight arrays (bf16)."""
    f = lambda a: np.asarray(a, np.float32)
    W1, a1s, a1d, b1 = f(W1), f(a1s), f(a1d), f(b1)
    W2, a2s, a2d, b2 = f(W2), f(a2s), f(a2d), f(b2)
    Wl, bl, Wr, M1, mb1, M2, mb2 = f(Wl), f(bl), f(Wr), f(M1), f(mb1), f(M2), f(mb2)

    def bd(a):  # [HEADS, HID] -> block diag [C, HEADS]
        out = np.zeros((C, HEADS), np.float32)
        for h in range(HEADS):
            out[h * HID:(h + 1) * HID, h] = a[h]
        return out

    w1cat = np.concatenate([W1, W1 @ bd(a1s), W1 @ bd(a1d)], 1)  # [Fin,198]
    w2cat = np.concatenate([W2, W2 @ bd(a2s), W2 @ bd(a2d)], 1)  # [C,198]
    wlmm = Wl @ M1 @ M2                                          # [C,16]
    wrmm = Wr @ M1 @ M2                                          # [C,16]
    cvec = bl @ M1 @ M2 + mb1 @ M2 + mb2                         # [16]
    bfc = lambda a: np.ascontiguousarray(a.astype(BF16))
    return dict(
        w1cat=bfc(w1cat),
        w2a=bfc(w2cat[0:P]),
        w2b=bfc(w2cat[P:C]),
        wla=bfc(wlmm[0:P]),
        wlb=bfc(wlmm[P:C]),
        wra=bfc(wrmm[0:P]),
        wrb=bfc(wrmm[P:C]),
        brep1=bfc(np.tile(b1[None, :], (P, 1))),
        brep2=bfc(np.tile(b2[None, :], (P, 1))),
        crep=np.ascontiguousarray(np.tile(cvec[None, :], (P, 1)).astype(np.float32)),
    )


def build_program(cfg, reps=1):
    """Build the Bass/Tile program (SPMD, identical across cores).

    reps>1 repeats the whole computation (for floor-free benchmarking)."""
    import concourse.bass as bass
    import concourse.bacc as bacc
    import concourse.mybir as mybir
    import concourse.tile as tile
    from concourse.masks import make_identity

    n_cores = cfg["n_cores"]
    NP_, NT_, Tg, Ts, Fin = cfg["NP"], cfg["NT"], cfg["T_gat"], cfg["T_sage"], cfg["Fin"]
    NG = n_cores * NP_
    f32 = mybir.dt.float32
    i32 = mybir.dt.int32
    bf = mybir.dt.bfloat16
    A = mybir.AluOpType
    ACT = mybir.ActivationFunctionType

    _ceil_i = lambda a, b: -(-a // b)

    nc = bacc.Bacc("TRN2", target_bir_lowering=False, num_devices=n_cores)

    # I/O
    xT = nc.dram_tensor("xT", [P, NG], bf, kind="ExternalInput")
    w1cat = nc.dram_tensor("w1cat", [Fin, ROWW], bf, kind="ExternalInput")
    w2a = nc.dram_tensor("w2a", [P, ROWW], bf, kind="ExternalInput")
    w2b = nc.dram_tensor("w2b", [C - P, ROWW], bf, kind="ExternalInput")
    wla = nc.dram_tensor("wla", [P, OUT_C], bf, kind="ExternalInput")
    wlb = nc.dram_tensor("wlb", [C - P, OUT_C], bf, kind="ExternalInput")
    wra = nc.dram_tensor("wra", [P, OUT_C], bf, kind="ExternalInput")
    wrb = nc.dram_tensor("wrb", [C - P, OUT_C], bf, kind="ExternalInput")
    brep1 = nc.dram_tensor("brep1", [P, C], bf, kind="ExternalInput")
    brep2 = nc.dram_tensor("brep2", [P, C], bf, kind="ExternalInput")
    crep = nc.dram_tensor("crep", [P, OUT_C], f32, kind="ExternalInput")
    meta_gat1 = nc.dram_tensor("meta_gat1", [NT_, P, Tg], i32, kind="ExternalInput")
    meta_gat2 = nc.dram_tensor("meta_gat2", [NT_, P, Tg], i32, kind="ExternalInput")
    slot_gat = nc.dram_tensor("slot_gat", [NT_, P, Tg], bf, kind="ExternalInput")
    slot_gat_r = nc.dram_tensor("slot_gat_r", [NT_, 1, Tg * P], bf,
                                kind="ExternalInput")
    sdeginv = nc.dram_tensor("sdeginv", [NT_, P, 1], f32, kind="ExternalInput")
    out_sh = nc.dram_tensor("out_sh", [NP_, OUT_C], f32, kind="ExternalOutput")

    g1_full = nc.dram_tensor("g1_full", [NG, ROWW], bf, kind="Internal")
    ald1_loc = nc.dram_tensor("ald1_loc", [NP_, 4], bf, kind="Internal")
    g2_loc = nc.dram_tensor("g2_loc", [NP_, ROWW], bf, kind="Internal")
    ald2_loc = nc.dram_tensor("ald2_loc", [NP_, 4], bf, kind="Internal")
    y_loc = nc.dram_tensor("y_loc", [NP_, OUT_C], bf, kind="Internal")
    yr_loc = nc.dram_tensor("yr_loc", [NP_, OUT_C], f32, kind="Internal")
    if n_cores > 1:
        g2_full = nc.dram_tensor("g2_full", [NG, ROWW], bf, kind="Internal",
                                 addr_space="Shared")
        y_full = nc.dram_tensor("y_full", [NG, OUT_C], bf, kind="Internal",
                                addr_space="Shared")
    else:
        g2_full, y_full = g2_loc, y_loc

    with tile.TileContext(nc) as tc:
        import contextlib
        ctx = contextlib.ExitStack()
        with ctx:
            cpool = ctx.enter_context(tc.tile_pool(name="const", bufs=1))
            dpool = ctx.enter_context(tc.tile_pool(name="dense", bufs=4))
            epool = ctx.enter_context(tc.tile_pool(name="edge", bufs=4))
            spool = ctx.enter_context(tc.tile_pool(name="spool", bufs=3))
            accps = ctx.enter_context(tc.tile_pool(name="accps", bufs=2, space="PSUM"))
            trps = ctx.enter_context(tc.tile_pool(name="trps", bufs=1, space="PSUM"))
            brps_p = ctx.enter_context(tc.tile_pool(name="brps", bufs=2, space="PSUM"))
            aldps_p = ctx.enter_context(tc.tile_pool(name="aldps", bufs=1, space="PSUM"))
            ops_ps = ctx.enter_context(tc.tile_pool(name="opsps", bufs=1, space="PSUM"))

            # constants
            iota_i = cpool.tile([P, P], i32)
            iota_b = cpool.tile([P, P], bf)
            nc.gpsimd.iota(iota_i[:], pattern=[[1, P]], base=0, channel_multiplier=0)
            nc.vector.tensor_copy(iota_b[:], iota_i[:])
            ident = cpool.tile([P, P], bf)
            make_identity(nc, ident[:])
            ones_sb = cpool.tile([1, P], bf)
            nc.vector.memset(ones_sb[:], 1.0)
            ipt_i = cpool.tile([P, 1], i32)
            ipt_b = cpool.tile([P, 1], bf)
            nc.gpsimd.iota(ipt_i[:], pattern=[[0, 1]], base=0, channel_multiplier=1)
            nc.vector.tensor_copy(ipt_b[:], ipt_i[:])

            w1sb = cpool.tile([Fin, ROWW], bf)
            nc.sync.dma_start(w1sb[:], w1cat[:, :])
            w2a_sb = cpool.tile([P, ROWW], bf)
            w2b_sb = cpool.tile([C - P, ROWW], bf)
            nc.sync.dma_start(w2a_sb[:], w2a[:, :])
            nc.sync.dma_start(w2b_sb[:], w2b[:, :])
            wla_sb = cpool.tile([P, OUT_C], bf)
            wlb_sb = cpool.tile([C - P, OUT_C], bf)
            wra_sb = cpool.tile([P, OUT_C], bf)
            wrb_sb = cpool.tile([C - P, OUT_C], bf)
            nc.sync.dma_start(wla_sb[:], wla[:, :])
            nc.sync.dma_start(wlb_sb[:], wlb[:, :])
            nc.sync.dma_start(wra_sb[:], wra[:, :])
            nc.sync.dma_start(wrb_sb[:], wrb[:, :])
            b1sb = cpool.tile([P, C], bf)
            b2sb = cpool.tile([P, C], bf)
            csb = cpool.tile([P, OUT_C], f32)
            nc.sync.dma_start(b1sb[:], brep1[:, :])
            nc.sync.dma_start(b2sb[:], brep2[:, :])
            nc.sync.dma_start(csb[:], crep[:, :])

            def dense1(scope):
                # replicated: g1 rows for ALL cores' nodes, written locally.
                # GS tiles per iteration to amortize DMA fixed costs.
                GS = 8
                with nc.named_scope(scope):
                    for c0 in range(0, NG // P, GS):
                        gs = min(GS, NG // P - c0)
                        xt = dpool.tile([P, GS * P], bf, tag="xt")
                        nc.sync.dma_start(xt[:, 0:gs * P],
                                          xT[:, c0 * P:(c0 + gs) * P])
                        gsb = dpool.tile([P, GS, ROWW], bf, tag="gsb")
                        n_ald = max(0, min(gs, NT_ - c0))
                        asb = None
                        if n_ald:
                            asb = dpool.tile([P, GS, 4], bf, tag="asb")
                        for i in range(gs):
                            ps = accps.tile([P, ROWW], f32, tag="acc")
                            nc.tensor.matmul(out=ps[:],
                                             lhsT=xt[:, i * P:(i + 1) * P],
                                             rhs=w1sb[:], start=True, stop=True)
                            nc.scalar.activation(gsb[:, i, :], ps[:], ACT.Copy)
                            if i < n_ald:
                                nc.scalar.activation(asb[:, i, :],
                                                     ps[:, C + 2:ROWW], ACT.Copy)
                        nc.sync.dma_start(
                            g1_full[c0 * P:(c0 + gs) * P, :].rearrange(
                                "(i p) w -> p i w", p=P),
                            gsb[:, 0:gs, :])
                        if n_ald:
                            nc.sync.dma_start(
                                ald1_loc[c0 * P:(c0 + n_ald) * P, :].rearrange(
                                    "(i p) w -> p i w", p=P),
                                asb[:, 0:n_ald, :])

            def edge_phase(meta_dram, g_dram, ald_dram, self_dram, b_sb, fused,
                           scope, mid_hook=None):
                TgP = Tg * P
                NB = _ceil_i(TgP, 512)
                with nc.named_scope(scope):
                    for t in range(NT_):
                        mi = epool.tile([P, Tg], i32, tag="mi")
                        nc.sync.dma_start(mi[:], meta_dram[t, :, :])
                        slt = epool.tile([P, Tg], bf, tag="slt")
                        nc.sync.dma_start(slt[:], slot_gat[t, :, :])
                        slr = epool.tile([1, TgP], bf, tag="slr")
                        nc.sync.dma_start(slr[:], slot_gat_r[t, :, :])
                        aldt = epool.tile([P, 4], bf, tag="aldt")
                        nc.sync.dma_start(aldt[:], ald_dram[t * P:(t + 1) * P, :])
                        G = epool.tile([P, Tg, ROWW], bf, tag="G")
                        for j in range(Tg):
                            nc.gpsimd.indirect_dma_start(
                                out=G[:, j, :], out_offset=None, in_=g_dram[:, :],
                                in_offset=bass.IndirectOffsetOnAxis(
                                    ap=mi[:, j:j + 1], axis=0))
                        S_all = spool.tile([P, Tg, P], bf, tag="S")
                        nc.vector.tensor_tensor(
                            out=S_all[:],
                            in0=iota_b[:].unsqueeze(1).broadcast_to([P, Tg, P]),
                            in1=slt[:].unsqueeze(2).broadcast_to([P, Tg, P]),
                            op=A.is_equal)
                        # dst-side attention terms routed via transposed one-hots:
                        # st_all[d, j*P+e] = (slot_e == d); alde = st^T @ aldt
                        st_all = spool.tile([P, TgP], bf, tag="st")
                        for g in range(NB):
                            lo = g * 512
                            hi = min(lo + 512, TgP)
                            brt = brps_p.tile([P, 512], f32, tag="br")
                            nc.tensor.matmul(out=brt[:, 0:hi - lo], lhsT=ones_sb[:],
                                             rhs=slr[:, lo:hi], start=True, stop=True)
                            brb = epool.tile([P, 512], bf, tag="brb")
                            nc.scalar.activation(brb[:, 0:hi - lo], brt[:, 0:hi - lo],
                                                 ACT.Copy)
                            nc.vector.tensor_tensor(
                                out=st_all[:, lo:hi],
                                in0=ipt_b[:].broadcast_to([P, hi - lo]),
                                in1=brb[:, 0:hi - lo], op=A.is_equal)
                        aldeps = aldps_p.tile([P, Tg * HEADS], f32, tag="alde")
                        for j in range(Tg):
                            nc.tensor.matmul(
                                out=aldeps[:, j * HEADS:(j + 1) * HEADS],
                                lhsT=st_all[:, j * P:(j + 1) * P],
                                rhs=aldt[:, 1:4], start=True, stop=True)
                        ab = epool.tile([P, Tg, HEADS], bf, tag="ab")
                        nc.scalar.activation(
                            ab[:].rearrange("p j h -> p (j h)"), aldeps[:], ACT.Copy)
                        att = epool.tile([P, Tg, HEADS], bf, tag="att")
                        nc.vector.tensor_tensor(out=att[:], in0=G[:, :, C:C + HEADS],
                                                in1=ab[:], op=A.add)
                        nc.scalar.activation(att[:], att[:], ACT.Lrelu, alpha=0.2)
                        nc.scalar.activation(att[:], att[:], ACT.Exp)
                        nc.vector.tensor_copy(G[:, :, C:C + HEADS], att[:])
                        gh = G[:, :, 0:C].rearrange("p j (h c) -> p j h c", h=HEADS)
                        nc.vector.tensor_tensor(
                            out=gh, in0=gh,
                            in1=att[:].to_broadcast([P, Tg, HEADS, HID]),
                            op=A.mult)
                        # self-loop contribution: rhs built from the tile's own
                        # rows (direct load), accumulated via an identity matmul
                        gt_self = epool.tile([P, ROWW], bf, tag="gself")
                        nc.sync.dma_start(gt_self[:],
                                          self_dram[t * P:(t + 1) * P, :])
                        att_s = epool.tile([P, HEADS], bf, tag="atts")
                        nc.vector.tensor_tensor(out=att_s[:],
                                                in0=gt_self[:, C:C + HEADS],
                                                in1=aldt[:, 1:4], op=A.add)
                        nc.scalar.activation(att_s[:], att_s[:], ACT.Lrelu, alpha=0.2)
                        nc.scalar.activation(att_s[:], att_s[:], ACT.Exp)
                        selfrhs = epool.tile([P, ROWW], bf, tag="selfrhs")
                        nc.vector.tensor_tensor(
                            out=selfrhs[:, 0:C].rearrange("p (h c) -> p h c", h=HEADS),
                            in0=gt_self[:, 0:C].rearrange("p (h c) -> p h c", h=HEADS),
                            in1=att_s[:].to_broadcast([P, HEADS, HID]), op=A.mult)
                        nc.vector.tensor_copy(selfrhs[:, C:C + HEADS], att_s[:])
                        nc.vector.tensor_copy(selfrhs[:, C + HEADS:ROWW], att_s[:])
                        ps = accps.tile([P, ROWW], f32, tag="acc")
                        for j in range(Tg):
                            nc.tensor.matmul(out=ps[:], lhsT=S_all[:, j, :],
                                             rhs=G[:, j, :],
                                             start=(j == 0), stop=False)
                        nc.tensor.matmul(out=ps[:], lhsT=ident[:], rhs=selfrhs[:],
                                         start=False, stop=True)
                        zinv = epool.tile([P, HEADS], f32, tag="zinv")
                        nc.vector.reciprocal(zinv[:], ps[:, C:C + HEADS])
                        osb = epool.tile([P, C], bf, tag="osb")
                        nc.vector.tensor_tensor(
                            out=osb[:].rearrange("p (h c) -> p h c", h=HEADS),
                            in0=ps[:, 0:C].rearrange("p (h c) -> p h c", h=HEADS),
                            in1=zinv[:].to_broadcast([P, HEADS, HID]),
                            op=A.mult)
                        nc.vector.tensor_tensor(out=osb[:], in0=osb[:], in1=b_sb[:],
                                                op=A.add)
                        if not fused:
                            f2sb = epool.tile([P, C], bf, tag="fout")
                            nc.scalar.activation(f2sb[:], osb[:], ACT.Relu)
                            tp1 = trps.tile([P, P], bf, tag="tp")
                            nc.tensor.transpose(out=tp1[:], in_=f2sb[:, 0:P],
                                                identity=ident[:])
                            ft1 = epool.tile([P, P], bf, tag="ft1")
                            nc.scalar.activation(ft1[:], tp1[:], ACT.Copy)
                            tp2 = trps.tile([P, P], bf, tag="tp")
                            nc.tensor.transpose(out=tp2[:C - P, :], in_=f2sb[:, P:C],
                                                identity=ident[:])
                            ft2 = epool.tile([C - P, P], bf, tag="ft2")
                            nc.scalar.activation(ft2[:], tp2[:C - P, :], ACT.Copy)
                            ps2 = accps.tile([P, ROWW], f32, tag="acc")
                            nc.tensor.matmul(out=ps2[:], lhsT=ft1[:], rhs=w2a_sb[:],
                                             start=True, stop=False)
                            nc.tensor.matmul(out=ps2[:], lhsT=ft2[:], rhs=w2b_sb[:],
                                             start=False, stop=True)
                            gsb = epool.tile([P, ROWW], bf, tag="g2sb")
                            nc.scalar.activation(gsb[:], ps2[:], ACT.Copy)
                            nc.sync.dma_start(g2_loc[t * P:(t + 1) * P, :], gsb[:])
                            asb = epool.tile([P, 4], bf, tag="a2sb")
                            nc.scalar.activation(asb[:], ps2[:, C + 2:ROWW], ACT.Copy)
                            nc.sync.dma_start(ald2_loc[t * P:(t + 1) * P, :], asb[:])
                        else:
                            f3t = epool.tile([P, C], bf, tag="fout")
                            nc.scalar.activation(f3t[:], osb[:], ACT.Relu)
                            tp1 = trps.tile([P, P], bf, tag="tp")
                            nc.tensor.transpose(out=tp1[:], in_=f3t[:, 0:P],
                                                identity=ident[:])
                            ft1 = epool.tile([P, P], bf, tag="ft1")
                            nc.scalar.activation(ft1[:], tp1[:], ACT.Copy)
                            tp2 = trps.tile([P, P], bf, tag="tp")
                            nc.tensor.transpose(out=tp2[:C - P, :], in_=f3t[:, P:C],
                                                identity=ident[:])
                            ft2 = epool.tile([C - P, P], bf, tag="ft2")
                            nc.scalar.activation(ft2[:], tp2[:C - P, :], ACT.Copy)
                            yps = ops_ps.tile([P, OUT_C], f32, tag="yps")
                            nc.tensor.matmul(out=yps[:], lhsT=ft1[:], rhs=wla_sb[:],
                                             start=True, stop=False)
                            nc.tensor.matmul(out=yps[:], lhsT=ft2[:], rhs=wlb_sb[:],
                                             start=False, stop=True)
                            yrps = ops_ps.tile([P, OUT_C], f32, tag="yrps")
                            nc.tensor.matmul(out=yrps[:], lhsT=ft1[:], rhs=wra_sb[:],
                                             start=True, stop=False)
                            nc.tensor.matmul(out=yrps[:], lhsT=ft2[:], rhs=wrb_sb[:],
                                             start=False, stop=True)
                            ysb = epool.tile([P, OUT_C], bf, tag="ysb")
                            nc.scalar.activation(ysb[:], yps[:], ACT.Copy)
                            nc.sync.dma_start(y_loc[t * P:(t + 1) * P, :], ysb[:])
                            yrsb = epool.tile([P, OUT_C], f32, tag="yrsb")
                            nc.vector.tensor_tensor(out=yrsb[:], in0=yrps[:],
                                                    in1=csb[:], op=A.add)
                            nc.sync.dma_start(yr_loc[t * P:(t + 1) * P, :], yrsb[:])
                        if mid_hook is not None:
                            mid_hook(t)

            def allgather(loc_ap, full_ap, scope):
                with nc.named_scope(scope):
                    nc.gpsimd.collective_compute(
                        "AllGather", A.bypass,
                        replica_groups=[list(range(n_cores))],
                        ins=[loc_ap],
                        outs=[full_ap],
                    )

            def sage(scope):
                with nc.named_scope(scope):
                    for t in range(NT_):
                        mi = epool.tile([P, Ts], i32, tag="smi")
                        nc.sync.dma_start(mi[:], meta_gat2[t, :, :])
                        slt = epool.tile([P, Ts], bf, tag="sslt")
                        nc.sync.dma_start(slt[:], slot_gat[t, :, :])
                        dg = epool.tile([P, 1], f32, tag="dg")
                        nc.sync.dma_start(dg[:], sdeginv[t, :, :])
                        Y = epool.tile([P, Ts, OUT_C], bf, tag="Y")
                        for j in range(Ts):
                            nc.gpsimd.indirect_dma_start(
                                out=Y[:, j, :], out_offset=None, in_=y_full[:, :],
                                in_offset=bass.IndirectOffsetOnAxis(
                                    ap=mi[:, j:j + 1], axis=0))
                        S_s = spool.tile([P, Ts, P], bf, tag="Ss")
                        nc.vector.tensor_tensor(
                            out=S_s[:],
                            in0=iota_b[:].unsqueeze(1).broadcast_to([P, Ts, P]),
                            in1=slt[:].unsqueeze(2).broadcast_to([P, Ts, P]),
                            op=A.is_equal)
                        ps = ops_ps.tile([P, OUT_C], f32, tag="yps")
                        for j in range(Ts):
                            nc.tensor.matmul(out=ps[:], lhsT=S_s[:, j, :],
                                             rhs=Y[:, j, :],
                                             start=(j == 0), stop=(j == Ts - 1))
                        agg = epool.tile([P, OUT_C], f32, tag="agg")
                        nc.vector.tensor_scalar(out=agg[:], in0=ps[:],
                                                scalar1=dg[:], scalar2=None,
                                                op0=A.mult)
                        yrt = epool.tile([P, OUT_C], f32, tag="yrt")
                        nc.sync.dma_start(yrt[:], yr_loc[t * P:(t + 1) * P, :])
                        nc.vector.tensor_tensor(out=agg[:], in0=agg[:], in1=yrt[:],
                                                op=A.add)
                        fin = epool.tile([P, OUT_C], f32, tag="fin")
                        nc.scalar.activation(fin[:], agg[:], ACT.Sigmoid)
                        nc.sync.dma_start(out_sh[t * P:(t + 1) * P, :], fin[:])

            # ---- program ----
            NT_A = (NT_ + 1) // 2
            HA = NT_A * P
            NG_A = n_cores * HA

            for rep in range(reps):
                r = f"_r{rep}" if reps > 1 else ""

                def ag2_hook(t, _r=r):
                    if n_cores <= 1:
                        return
                    if t == NT_A - 1:
                        allgather(g2_loc[0:HA, :], g2_full[0:NG_A, :], f"ag2a{_r}")
                    elif t == NT_ - 1:
                        allgather(g2_loc[HA:NP_, :], g2_full[NG_A:NG, :], f"ag2b{_r}")

                def agy_hook(t, _r=r):
                    if n_cores <= 1:
                        return
                    if t == NT_A - 1:
                        allgather(y_loc[0:HA, :], y_full[0:NG_A, :], f"agya{_r}")
                    elif t == NT_ - 1:
                        allgather(y_loc[HA:NP_, :], y_full[NG_A:NG, :], f"agyb{_r}")

                dense1(f"dense1{r}")
                edge_phase(meta_gat1, g1_full, ald1_loc, g1_full, b1sb, False,
                           f"edge1{r}", mid_hook=ag2_hook)
                edge_phase(meta_gat2, g2_full, ald2_loc, g2_loc, b2sb, True,
                           f"edge2{r}", mid_hook=agy_hook)
                sage(f"sage{r}")

    nc.compile()
    return nc


LAST_RESULTS = None


def kernel(**inputs):
    global LAST_RESULTS
    import os
    x = np.asarray(inputs["x"], np.float32)
    edge_index = np.asarray(inputs["edge_index"])
    cfg, cores, ggid = preprocess(x, edge_index, N, NCORES)
    wts = fold_weights(
        inputs["W1"], inputs["a1s"], inputs["a1d"], inputs["b1"],
        inputs["W2"], inputs["a2s"], inputs["a2d"], inputs["b2"],
        inputs["Wl"], inputs["bl"], inputs["Wr"],
        inputs["M1"], inputs["mb1"], inputs["M2"], inputs["mb2"])
    nc = build_program(cfg)
    in_maps = [dict(core, **wts) for core in cores]

    from concourse import bass_utils
    res = bass_utils.run_bass_kernel_spmd(
        nc, in_maps, core_ids=list(range(NCORES)),
        trace=bool(int(os.environ.get("GAT_TRACE", "0"))))
    LAST_RESULTS = res
    NPp = cfg["NP"]
    out = np.zeros((N, OUT_C), np.float32)
    for k in range(NCORES):
        o = res.results[k]["out_sh"]
        lo, hi = k * cfg["NPC"], (k + 1) * cfg["NPC"]
        out[lo:hi] = o[ggid[lo:hi] - k * NPp]
    return out
